# revision 7
# baseline (speedup 1.0000x reference)
"""MLA (multi-head latent attention) Trainium2 kernel, 8-core SPMD, v2.

Design (vs the v1 2x4-grid kernel):
  - 8-way head sharding: core c owns heads [4c, 4c+4), all 2048 rows.
  - NO collectives (the AllGather cost ~150-250us on this axon setup).
    Instead the low-rank projections are fused on the host:
      W_q = w_dq @ w_uq, W_qr = w_dq @ w_rq, W_k = w_dkv @ w_uk,
      W_v = w_dkv @ w_uv, so Q/Qr/K/V/Kr come from single matmuls
      against seq directly.
  - fp8e4 DoubleRow matmuls (measured ~3.5x bf16 per unit work) for all
    Q/K/V/Kr builds, scores, AV and softmax denominator. Out-projection
    stays bf16 (fp8 there costs ~6% rel err). End-to-end sim rel err
    ~1.3e-2 (budget 2e-2).
  - Fused weights are scaled by SC=64 on host to clear fp8e4's subnormal
    range; compensated in the PSUM->fp8 conversion scales.
  - Bias algebra: b_uk, b_rk and the C_KV bias' K-contribution are
    dropped (constant-per-query scores shifts, softmax-invariant);
    V-path bias is folded into a host-side b_out_eff = b_out + b_v@w_out.
  - Phase C is qb-major with the out-projection (phase D) of the
    previous qb interleaved one PSUM-chain per (head, kt-pair) slot, so
    D hides under the scalar-engine exp stream (the critical path).
  - Partial outputs are DMA'd f32 straight from PSUM; host sums the 8
    head-group partials.
"""

import numpy as np
import ml_dtypes

import jax
from jax.sharding import Mesh, PartitionSpec, NamedSharding
try:
    from jax.experimental.shard_map import shard_map
except ImportError:  # newer jax
    from jax import shard_map

import concourse.tile as tile
from concourse import bacc, mybir
from concourse import bass2jax

BF16 = mybir.dt.bfloat16
F16 = mybir.dt.float16
F32 = mybir.dt.float32
FP8 = mybir.dt.float8e4
NP8 = ml_dtypes.float8_e4m3
AFT = mybir.ActivationFunctionType
ALU = mybir.AluOpType
DR = mybir.MatmulPerfMode.DoubleRow

# problem dims
S, DE, DR_DIM, H, DH, DM = 2048, 4096, 64, 32, 128, 4096
GH = 4                  # heads per core
QB = 512                # query block
SCALER = float(1.0 / np.sqrt(np.float32(DH + DR_DIM)))
SC = 64.0               # fused-weight prescale (fp8 subnormal avoidance)
SC2 = 32.0              # up-projection prescale (KV two-step path)
P = 128


def _emit_body(nc, tc, t):
    from contextlib import ExitStack

    with ExitStack() as ctx:
        cp = ctx.enter_context(tc.tile_pool(name="persist", bufs=1))

        # den matmul uses 1/SC instead of 1.0 (exact in fp8e4): psD = den/SC,
        # so the reciprocal comes out as SC/den and the OT multiply lands at
        # SC*OT -- the scale the fp8 hi/lo out-projection split needs, free.
        ones8 = cp.tile([P, 2, P], FP8, tag="ones8", name="ones8")
        nc.any.memset(ones8[:], 1.0 / SC)

        # Heads 0,2 keep their rotary dims in partition rows 64:128; heads
        # 1,3 in rows 0:64 (they come out of packed M=128 Qr/Kr builds and
        # engines cannot shift partitions). Kr is built in both halves.
        UPPER = (0, 2)
        Kr8 = cp.tile([P, S], FP8, tag="kr8", name="Kr8")
        KKr = cp.tile([P, GH, 16, 2, P], FP8, tag="kkr", name="KKr")
        QQr = cp.tile([P, GH, 2, S], FP8, tag="qqr", name="QQr")
        V_G = cp.tile([P, 16, 512], FP8, tag="vg", name="VG")
        OThi = cp.tile([P, GH, S], FP8, tag="othi", name="OThi")
        OTlo = cp.tile([P, GH, S], FP8, tag="otlo", name="OTlo")
        # zero the rotary padding rows of the DoubleRow slot-1
        for h in range(GH):
            pad = slice(0, DR_DIM) if h in UPPER else slice(DR_DIM, P)
            nc.any.memset(KKr[pad, h, :, 1, :], 0.0)
            nc.any.memset(QQr[pad, h, 1, :], 0.0)

        # ---------------- pre-phase: K / Kr / V / Q builds (blk-major) ----
        # seqT, wq, bq persist into phase C (Q builds for qb>=1 are emitted
        # just-in-time inside phase C to fill its act-bound slack)
        LO, HI = slice(0, DR_DIM), slice(DR_DIM, P)
        seqp = ctx.enter_context(tc.tile_pool(name="seqp", bufs=1))
        seqT = seqp.tile([P, 32, S], FP8, tag="seqT", name="seqT")
        wq_tiles = []
        for h in range(GH):
            wq = seqp.tile([P, 32, P], FP8, tag="wq", bufs=4, name=f"wq{h}")
            wq_tiles.append(wq)
        bq = seqp.tile([P, GH], F32, tag="bq", name="bq")
        with tc.tile_pool(name="pre_ps", bufs=4, space="PSUM") as pp, \
             tc.tile_pool(name="pre_in", bufs=1) as pin:
            # wdkv rides first on gpsimd so the first C_KV chain starts ASAP;
            # seq blk0 split across sync+scalar
            wdkv = pin.tile([P, 32, 512], FP8, tag="wdkv", name="wdkv")
            for q in range(4):
                nc.gpsimd.dma_start(wdkv[:, 8 * q:8 * (q + 1), :],
                                    t["wdkv"][:, 8 * q:8 * (q + 1), :])
            for i, eng in enumerate([nc.sync, nc.scalar, nc.sync, nc.scalar]):
                eng.dma_start(seqT[:, i * 8:(i + 1) * 8, 0:QB],
                              t["seqT"][0, :, i * 8:(i + 1) * 8, :])
            wuk = pin.tile([P, 4, 512], FP8, tag="wuk", name="wuk")
            wuv = pin.tile([P, 4, 512], FP8, tag="wuv", name="wuv")
            nc.gpsimd.dma_start(wuk[:], t["wuk"][:])
            nc.gpsimd.dma_start(wuv[:], t["wuv"][:])
            wqrk = pin.tile([P, 3, 32, P], FP8, tag="wqrk", name="wqrk")
            for pk in range(3):
                nc.gpsimd.dma_start(wqrk[:, pk], t["wqrk"][pk])
            for blk, eng in [(1, nc.scalar), (2, nc.sync), (3, nc.scalar)]:
                eng.dma_start(seqT[:, :, blk * QB:(blk + 1) * QB],
                              t["seqT"][blk])
            nc.gpsimd.dma_start(bq[:], t["bq"][:])
            bqr = pin.tile([P, GH], F32, tag="bqr", name="bqr")
            nc.gpsimd.dma_start(bqr[:], t["bqr"][:])
            for h in range(GH):
                nc.sync.dma_start(wq_tiles[h][:], t["wq"][h])
            # C8 = fp8(SC * C_KV^T) [latent 512 as 4x128, keys 2048]
            C8 = pin.tile([P, 4, S], FP8, tag="c8", name="C8")
            for blk in range(4):
                sl = slice(blk * QB, (blk + 1) * QB)
                # latent C_KV^T for this key blk
                for lt in range(4):
                    ps = pp.tile([P, QB], F32, tag="ps", name=f"psC{lt}_{blk}")
                    for j in range(16):
                        nc.tensor.matmul(ps[:],
                                         wdkv[:, 2 * j:2 * j + 2,
                                              lt * P:(lt + 1) * P],
                                         seqT[:, 2 * j:2 * j + 2, sl],
                                         start=(j == 0), stop=(j == 15),
                                         perf_mode=DR)
                    nc.scalar.activation(C8[:, lt, sl], ps[:], AFT.Identity,
                                         scale=1.0)
                # K per head from latent: KKr[:, h, kt, 0, :] = K^T[kdim, keys]
                for h in range(GH):
                    ps = pp.tile([P, QB], F32, tag="ps", name=f"psK{h}_{blk}")
                    for u in range(2):
                        nc.tensor.matmul(ps[:],
                                         wuk[:, 2 * u:2 * u + 2,
                                             h * P:(h + 1) * P],
                                         C8[:, 2 * u:2 * u + 2, sl],
                                         start=(u == 0), stop=(u == 1),
                                         perf_mode=DR)
                    nc.vector.tensor_scalar_mul(
                        KKr[:, h, blk * 4:(blk + 1) * 4, 0, :], ps[:],
                        1.0 / (SC * SC2))
                # V from latent: V_G[:, kt, :] = V[keys 128, 512 vdims]
                for k4 in range(4):
                    kt = blk * 4 + k4
                    ps = pp.tile([P, 512], F32, tag="ps", name=f"psV{kt}")
                    for u in range(2):
                        nc.tensor.matmul(ps[:],
                                         C8[:, 2 * u:2 * u + 2,
                                            kt * P:(kt + 1) * P],
                                         wuv[:, 2 * u:2 * u + 2, :],
                                         start=(u == 0), stop=(u == 1),
                                         perf_mode=DR)
                    nc.vector.tensor_scalar_mul(V_G[:, kt, :], ps[:],
                                                1.0 / (SC * SC2))
                # packed Qr/Kr builds: [Kr|Qr_h0], [Qr_h1|Qr_h2], [Qr_h3|Kr]
                for pk in range(3):
                    ps = pp.tile([P, QB], F32, tag="ps", name=f"psP{pk}_{blk}")
                    for j in range(16):
                        nc.tensor.matmul(ps[:], wqrk[:, pk, 2 * j:2 * j + 2, :],
                                         seqT[:, 2 * j:2 * j + 2, sl],
                                         start=(j == 0), stop=(j == 15),
                                         perf_mode=DR)
                    if pk == 0:
                        nc.vector.tensor_scalar_mul(Kr8[LO, sl], ps[LO, :],
                                                    1.0 / SC)
                        nc.scalar.activation(QQr[HI, 0, 1, sl], ps[HI, :],
                                             AFT.Identity,
                                             bias=bqr[HI, 0:1],
                                             scale=SCALER / SC)
                    elif pk == 1:
                        nc.scalar.activation(QQr[LO, 1, 1, sl], ps[LO, :],
                                             AFT.Identity,
                                             bias=bqr[LO, 1:2],
                                             scale=SCALER / SC)
                        nc.scalar.activation(QQr[HI, 2, 1, sl], ps[HI, :],
                                             AFT.Identity,
                                             bias=bqr[HI, 2:3],
                                             scale=SCALER / SC)
                    else:
                        nc.scalar.activation(QQr[LO, 3, 1, sl], ps[LO, :],
                                             AFT.Identity,
                                             bias=bqr[LO, 3:4],
                                             scale=SCALER / SC)
                        nc.vector.tensor_scalar_mul(Kr8[HI, sl], ps[HI, :],
                                                    1.0 / SC)
                # Q content per head -- only qb0 up front; the rest are
                # built just-in-time inside phase C
                if blk == 0:
                    for h in range(GH):
                        ps = pp.tile([P, QB], F32, tag="ps",
                                     name=f"psQ{h}_{blk}")
                        for j in range(16):
                            nc.tensor.matmul(ps[:],
                                             wq_tiles[h][:, 2 * j:2 * j + 2, :],
                                             seqT[:, 2 * j:2 * j + 2, sl],
                                             start=(j == 0), stop=(j == 15),
                                             perf_mode=DR)
                        nc.scalar.activation(QQr[:, h, 0, sl], ps[:],
                                             AFT.Identity, bias=bq[:, h:h + 1],
                                             scale=SCALER / SC)
            # replicate Kr into each head's slot-1 rows
            for h in range(GH):
                half = HI if h in UPPER else LO
                nc.vector.tensor_copy(KKr[half, h, :, 1, :], Kr8[half, :])

        # ---------------- phase C (+ interleaved phase D) ----------------
        pss = ctx.enter_context(tc.tile_pool(name="pss", bufs=2, space="PSUM"))
        pod = ctx.enter_context(tc.tile_pool(name="pod", bufs=2, space="PSUM"))
        pou = ctx.enter_context(tc.tile_pool(name="pou", bufs=2, space="PSUM"))
        pc = ctx.enter_context(tc.tile_pool(name="pc", bufs=1))
        pdw = ctx.enter_context(tc.tile_pool(name="pdw", bufs=1))

        wout_cur = [None, None]

        def load_wout(nt, slot):
            w = pdw.tile([P, 2, GH, 512], FP8, tag="wout", bufs=3,
                         name=f"wout{slot}_{nt}")
            for lvl in range(2):
                nc.gpsimd.dma_start(w[:, lvl], t["wout"][lvl, nt])
            wout_cur[slot % 2] = (nt, w)
            return w

        load_wout(0, 0)

        d_jobs = []     # pending (qb, nt, qt) out-projection chains

        def emit_d_job():
            if not d_jobs:
                return
            qb, nt, qt = d_jobs.pop(0)
            w = None
            for ent in wout_cur:
                if ent is not None and ent[0] == nt:
                    w = ent[1]
            if w is None:
                w = load_wout(nt, nt)
            ps = pou.tile([P, 512], F32, tag="po", name=f"psOut{qb}_{nt}_{qt}")
            qsl = slice(qb * QB + qt * P, qb * QB + (qt + 1) * P)
            terms = [(OThi, 0), (OThi, 1), (OTlo, 0)]
            for i, (ot, lvl) in enumerate(terms):
                for hp in range(2):
                    nc.tensor.matmul(ps[:], ot[:, 2 * hp:2 * hp + 2, qsl],
                                     w[:, lvl, 2 * hp:2 * hp + 2, :],
                                     start=(i == 0 and hp == 0),
                                     stop=(i == 2 and hp == 1),
                                     perf_mode=DR)
            osb = pdw.tile([P, 512], F16, tag="osb", bufs=4,
                           name=f"osb{qb}_{nt}_{qt}")
            nc.vector.tensor_scalar_mul(osb[:], ps[:], 1.0 / (SC * SC))
            nc.sync.dma_start(
                t["pout"][qb * QB + qt * P: qb * QB + (qt + 1) * P,
                          nt * 512:(nt + 1) * 512],
                osb[:])
            # prefetch next nt's weights at nt boundaries
            if qt == 0 and nt < 7 and all(
                    x is None or x[0] != nt + 1 for x in wout_cur):
                load_wout(nt + 1, nt + 1)

        for qb in range(4):
            for h in range(GH):
                PT = pc.tile([P, 16, QB], FP8, tag="pt", bufs=2,
                             name=f"pt{qb}_{h}")
                psO = pod.tile([P, QB], F32, tag="pod", name=f"psO{qb}_{h}")
                psD = pod.tile([P, QB], F32, tag="pod", name=f"psD{qb}_{h}")
                for tt in range(8):
                    psS = pss.tile([P, 2, QB], F32, tag="pss",
                                   name=f"psS{qb}_{h}_{tt}")
                    for u in range(2):
                        kt = 2 * tt + u
                        nc.tensor.matmul(psS[:, u, :], KKr[:, h, kt, :, :],
                                         QQr[:, h, :, qb * QB:(qb + 1) * QB],
                                         start=True, stop=True, perf_mode=DR)
                    nc.scalar.activation(PT[:, 2 * tt:2 * tt + 2, :],
                                         psS[:, :, :], AFT.Exp)
                    # the D chain needs no exp result: run it while the
                    # scalar engine finishes exp(tt-1), then AV can proceed
                    emit_d_job()
                    if tt > 0:
                        j = tt - 1
                        nc.tensor.matmul(psO[:],
                                         V_G[:, 2 * j:2 * j + 2,
                                             h * P:(h + 1) * P],
                                         PT[:, 2 * j:2 * j + 2, :],
                                         start=(j == 0), stop=False,
                                         perf_mode=DR)
                        nc.tensor.matmul(psD[:], ones8[:],
                                         PT[:, 2 * j:2 * j + 2, :],
                                         start=(j == 0), stop=False,
                                         perf_mode=DR)
                if qb < 3:
                    # JIT Q-content build for the next query block; emitted
                    # before the final AV pair so the PE has work while the
                    # scalar engine finishes exp(7)
                    nsl = slice((qb + 1) * QB, (qb + 2) * QB)
                    psQ = pou.tile([P, QB], F32, tag="po",
                                   name=f"psQj{qb}_{h}")
                    for j in range(16):
                        nc.tensor.matmul(psQ[:],
                                         wq_tiles[h][:, 2 * j:2 * j + 2, :],
                                         seqT[:, 2 * j:2 * j + 2, nsl],
                                         start=(j == 0), stop=(j == 15),
                                         perf_mode=DR)
                    nc.scalar.activation(QQr[:, h, 0, nsl], psQ[:],
                                         AFT.Identity, bias=bq[:, h:h + 1],
                                         scale=SCALER / SC)
                nc.tensor.matmul(psO[:], V_G[:, 14:16, h * P:(h + 1) * P],
                                 PT[:, 14:16, :], start=False, stop=True,
                                 perf_mode=DR)
                nc.tensor.matmul(psD[:], ones8[:], PT[:, 14:16, :],
                                 start=False, stop=True, perf_mode=DR)
                rcp = pc.tile([P, QB], F32, tag="rcp", bufs=2,
                              name=f"rcp{qb}_{h}")
                nc.vector.reciprocal(rcp[:], psD[:])
                # rcp = SC/den, so ots = SC*OT; split into fp8 hi + residual
                sl_q = slice(qb * QB, (qb + 1) * QB)
                ots = pc.tile([P, QB], BF16, tag="ots", bufs=2,
                              name=f"ots{qb}_{h}")
                nc.vector.tensor_tensor(ots[:], psO[:], rcp[:], ALU.mult)
                nc.vector.tensor_copy(OThi[:, h, sl_q], ots[:])
                nc.vector.tensor_tensor(OTlo[:, h, sl_q], ots[:],
                                        OThi[:, h, sl_q], ALU.subtract)
            # queue this qb's out-projection; it interleaves into qb+1
            for nt in range(8):
                for qt in range(4):
                    d_jobs.append((qb, nt, qt))
        while d_jobs:
            emit_d_job()


def _build_program(rep=1):
    nc = bacc.Bacc("TRN2", target_bir_lowering=False, debug=False)
    t = {}
    t["seqT"] = nc.dram_tensor("t_seqT", [4, P, 32, QB], FP8,
                               kind="ExternalInput")
    t["wq"] = nc.dram_tensor("t_wq", [GH, P, 32, P], FP8, kind="ExternalInput")
    t["wqrk"] = nc.dram_tensor("t_wqrk", [3, P, 32, P], FP8,
                               kind="ExternalInput")
    t["wdkv"] = nc.dram_tensor("t_wdkv", [P, 32, 512], FP8,
                               kind="ExternalInput")
    t["wuk"] = nc.dram_tensor("t_wuk", [P, 4, 512], FP8, kind="ExternalInput")
    t["wuv"] = nc.dram_tensor("t_wuv", [P, 4, 512], FP8, kind="ExternalInput")
    t["bq"] = nc.dram_tensor("t_bq", [P, GH], F32, kind="ExternalInput")
    t["bqr"] = nc.dram_tensor("t_bqr", [P, GH], F32, kind="ExternalInput")
    t["wout"] = nc.dram_tensor("t_wout", [2, 8, P, GH, 512], FP8,
                               kind="ExternalInput")
    t["pout"] = nc.dram_tensor("t_pout", [S, DM], F16, kind="ExternalOutput")

    with tile.TileContext(nc) as tc:
        for _ in range(rep):
            _emit_body(nc, tc, t)
    nc.compile()
    return nc


def _prep_in_maps(inputs):
    f32 = np.float32
    seq = np.asarray(inputs["sequence"], dtype=f32)[0]          # [2048, 4096]
    w_dq = np.asarray(inputs["w_dq"], dtype=f32)
    b_dq = np.asarray(inputs["b_dq"], dtype=f32)
    w_uq = np.asarray(inputs["w_uq"], dtype=f32)
    b_uq = np.asarray(inputs["b_uq"], dtype=f32)
    w_dkv = np.asarray(inputs["w_dkv"], dtype=f32)
    b_dkv = np.asarray(inputs["b_dkv"], dtype=f32)
    w_uk = np.asarray(inputs["w_uk"], dtype=f32)
    w_uv = np.asarray(inputs["w_uv"], dtype=f32)
    b_uv = np.asarray(inputs["b_uv"], dtype=f32)
    w_rq = np.asarray(inputs["w_rq"], dtype=f32)
    b_rq = np.asarray(inputs["b_rq"], dtype=f32)
    w_rk = np.asarray(inputs["w_rk"], dtype=f32)
    w_out = np.asarray(inputs["w_out"], dtype=f32)

    W_q = (w_dq @ w_uq) * SC                                    # [4096, 4096]
    W_qr = (w_dq @ w_rq) * SC                                   # [4096, 2048]
    W_kr = w_rk * SC                                            # [4096, 64]
    b_q = (b_dq @ w_uq + b_uq) * SCALER                         # [4096]
    b_qr = (b_dq @ w_rq + b_rq) * SCALER                        # [2048]

    seqT = np.ascontiguousarray(
        seq.reshape(4, QB, 32, P).transpose(0, 3, 2, 1)
    ).astype(NP8)                                               # [4,128,32,512]

    def tile32(w):  # [4096, n] -> [128, 32, n]
        return np.ascontiguousarray(
            w.reshape(32, P, w.shape[1]).transpose(1, 0, 2)).astype(NP8)

    def tile4(w):  # [512, n] -> [128, 4, n]
        return np.ascontiguousarray(
            w.reshape(4, P, w.shape[1]).transpose(1, 0, 2)).astype(NP8)

    shared = {"seqT": seqT, "wdkv": tile32(w_dkv * SC)}
    in_maps = []
    for c in range(8):
        cols = slice(c * GH * DH, (c + 1) * GH * DH)            # 512 cols
        colr = slice(c * GH * DR_DIM, (c + 1) * GH * DR_DIM)    # 256 cols
        m = dict(shared)
        m["wq"] = np.ascontiguousarray(
            W_q[:, cols].reshape(32, P, GH, DH).transpose(2, 1, 0, 3)
        ).astype(NP8)
        # packed Qr/Kr stationary tiles: [Kr|Qr_h0], [Qr_h1|Qr_h2], [Qr_h3|Kr]
        qr = [W_qr[:, colr][:, h * DR_DIM:(h + 1) * DR_DIM] for h in range(GH)]
        m["wqrk"] = np.stack([
            tile32(np.hstack([W_kr, qr[0]])),
            tile32(np.hstack([qr[1], qr[2]])),
            tile32(np.hstack([qr[3], W_kr])),
        ])
        m["wuk"] = tile4(w_uk[:, cols] * SC2)
        m["wuv"] = tile4(w_uv[:, cols] * SC2)
        m["bq"] = np.ascontiguousarray(
            b_q[cols].reshape(GH, P).T, dtype=f32)
        bqr_h = b_qr[colr].reshape(GH, DR_DIM).T                # [64, GH]
        m["bqr"] = np.ascontiguousarray(
            np.vstack([bqr_h, bqr_h]), dtype=f32)               # [128, GH]
        w64 = np.ascontiguousarray(
            (w_out[cols, :] * SC).reshape(GH, P, 8, 512).transpose(2, 1, 0, 3))
        whi = w64.astype(NP8)
        wlo = (w64 - whi.astype(f32)).astype(NP8)
        m["wout"] = np.stack([whi, wlo])
        in_maps.append({f"t_{k}": v for k, v in m.items()})
    return in_maps


class _Runner:
    """Cached sharded PJRT executor for a compiled Bass program."""

    def __init__(self, nc):
        bass2jax.install_neuronx_cc_hook()
        self.nc = nc
        in_names, out_names, out_avals = [], [], []
        pid_name = nc.partition_id_tensor.name if nc.partition_id_tensor else None
        for alloc in nc.m.functions[0].allocations:
            if not isinstance(alloc, mybir.MemoryLocationSet):
                continue
            name = alloc.memorylocations[0].name
            if alloc.kind == "ExternalInput":
                if name != pid_name:
                    in_names.append(name)
            elif alloc.kind == "ExternalOutput":
                out_names.append(name)
                shape = tuple(alloc.tensor_shape)
                dtype = mybir.dt.np(alloc.dtype)
                out_avals.append(jax.core.ShapedArray(shape, dtype))
        self.in_names = in_names
        self.out_names = out_names
        all_in_names = list(in_names) + list(out_names)
        if pid_name is not None:
            all_in_names.append(pid_name)

        def _body(*args):
            operands = list(args)
            if nc.partition_id_tensor is not None:
                operands.append(bass2jax.partition_id_tensor())
            outs = bass2jax._bass_exec_p.bind(
                *operands,
                out_avals=tuple(out_avals),
                in_names=tuple(all_in_names),
                out_names=tuple(out_names),
                lowering_input_output_aliases=(),
                sim_require_finite=True,
                sim_require_nnan=True,
                nc=nc,
            )
            return tuple(outs)

        devices = jax.devices()[:8]
        self.mesh = Mesh(np.asarray(devices), ("core",))
        n_io = len(in_names) + len(out_names)
        self.fn = jax.jit(
            shard_map(_body, mesh=self.mesh,
                      in_specs=(PartitionSpec("core"),) * n_io,
                      out_specs=(PartitionSpec("core"),) * len(out_names),
                      check_rep=False),
            keep_unused=True)
        self.sharding = NamedSharding(self.mesh, PartitionSpec("core"))
        self.dev_zero = [
            jax.device_put(
                np.zeros((8 * av.shape[0], *av.shape[1:]), av.dtype),
                self.sharding)
            for av in out_avals]
        self.out_avals = out_avals

    def stage(self, in_maps):
        dev_in = []
        for name in self.in_names:
            cat = np.concatenate([np.asarray(m[name]) for m in in_maps],
                                 axis=0)
            dev_in.append(jax.device_put(cat, self.sharding))
        return dev_in

    def run_staged(self, dev_in):
        outs = self.fn(*dev_in, *self.dev_zero)
        jax.block_until_ready(outs)
        return outs

    def run(self, in_maps):
        outs = self.run_staged(self.stage(in_maps))
        res = []
        for c in range(8):
            d = {}
            for i, name in enumerate(self.out_names):
                av = self.out_avals[i]
                d[name] = np.asarray(outs[i]).reshape(8, *av.shape)[c]
            res.append(d)
        return res


_CTX = None


def _get_ctx():
    global _CTX
    if _CTX is None:
        nc = _build_program(rep=1)
        _CTX = _Runner(nc)
    return _CTX


def kernel(**inputs):
    runner = _get_ctx()
    in_maps = _prep_in_maps(inputs)
    res = runner.run(in_maps)
    b_out = np.asarray(inputs["b_out"], dtype=np.float32)
    b_dkv = np.asarray(inputs["b_dkv"], dtype=np.float32)
    w_uv = np.asarray(inputs["w_uv"], dtype=np.float32)
    b_uv = np.asarray(inputs["b_uv"], dtype=np.float32)
    w_out = np.asarray(inputs["w_out"], dtype=np.float32)
    b_v = b_dkv @ w_uv + b_uv
    b_out_eff = b_out + b_v @ w_out
    out = res[0]["t_pout"].astype(np.float32)
    for c in range(1, 8):
        out = out + res[c]["t_pout"].astype(np.float32)
    out = out + b_out_eff
    return out.reshape(1, S, DM)



# revision 57
# speedup vs baseline: 1.9865x; 1.9865x over previous
"""MLA (multi-head latent attention) Trainium2 kernel, 8-core SPMD, v2.

Design (vs the v1 2x4-grid kernel):
  - 8-way head sharding: core c owns heads [4c, 4c+4), all 2048 rows.
  - NO collectives (the AllGather cost ~150-250us on this axon setup).
    Instead the low-rank projections are fused on the host:
      W_q = w_dq @ w_uq, W_qr = w_dq @ w_rq, W_k = w_dkv @ w_uk,
      W_v = w_dkv @ w_uv, so Q/Qr/K/V/Kr come from single matmuls
      against seq directly.
  - fp8e4 DoubleRow matmuls (measured ~3.5x bf16 per unit work) for all
    Q/K/V/Kr builds, scores, AV and softmax denominator. Out-projection
    stays bf16 (fp8 there costs ~6% rel err). End-to-end sim rel err
    ~1.3e-2 (budget 2e-2).
  - Fused weights are scaled by SC=64 on host to clear fp8e4's subnormal
    range; compensated in the PSUM->fp8 conversion scales.
  - Bias algebra: b_uk, b_rk and the C_KV bias' K-contribution are
    dropped (constant-per-query scores shifts, softmax-invariant);
    V-path bias is folded into a host-side b_out_eff = b_out + b_v@w_out.
  - Phase C is qb-major with the out-projection (phase D) of the
    previous qb interleaved one PSUM-chain per (head, kt-pair) slot, so
    D hides under the scalar-engine exp stream (the critical path).
  - Partial outputs are DMA'd f32 straight from PSUM; host sums the 8
    head-group partials.
"""

import numpy as np
import ml_dtypes

import jax
from jax.sharding import Mesh, PartitionSpec, NamedSharding
try:
    from jax.experimental.shard_map import shard_map
except ImportError:  # newer jax
    from jax import shard_map

import concourse.tile as tile
from concourse import bacc, mybir
from concourse import bass2jax

BF16 = mybir.dt.bfloat16
F16 = mybir.dt.float16
F32 = mybir.dt.float32
FP8 = mybir.dt.float8e4
NP8 = ml_dtypes.float8_e4m3
AFT = mybir.ActivationFunctionType
ALU = mybir.AluOpType
DR = mybir.MatmulPerfMode.DoubleRow

# problem dims
S, DE, DR_DIM, H, DH, DM = 2048, 4096, 64, 32, 128, 4096
GH = 4                  # heads per core
QB = 512                # query block
SCALER = float(1.0 / np.sqrt(np.float32(DH + DR_DIM)))
SC = 64.0               # fused-weight prescale (fp8 subnormal avoidance)
SC2 = 32.0              # up-projection prescale (KV two-step path)
P = 128


def _emit_body(nc, tc, t):
    from contextlib import ExitStack

    with ExitStack() as ctx:
        cp = ctx.enter_context(tc.tile_pool(name="persist", bufs=1))

        # den matmul uses 1/SC instead of 1.0 (exact in fp8e4): psD = den/SC,
        # so the reciprocal comes out as SC/den and the OT multiply lands at
        # SC*OT -- the scale the fp8 hi/lo out-projection split needs, free.
        ones8 = cp.tile([P, 2, P], FP8, tag="ones8", name="ones8")
        nc.any.memset(ones8[:], 1.0 / SC)

        # Heads 0,2 keep their rotary dims in partition rows 64:128; heads
        # 1,3 in rows 0:64 (they come out of packed M=128 Qr/Kr builds and
        # engines cannot shift partitions). Kr is built in both halves.
        UPPER = (0, 2)
        Kr8 = cp.tile([P, S], FP8, tag="kr8", name="Kr8")
        KKr = cp.tile([P, GH, 16, 2, P], FP8, tag="kkr", name="KKr")
        QQr = cp.tile([P, GH, 2, S], FP8, tag="qqr", name="QQr")
        V_G = cp.tile([P, 16, 512], FP8, tag="vg", name="VG")
        OThi = cp.tile([P, GH, S], FP8, tag="othi", name="OThi")
        OTlo = cp.tile([P, GH, S], FP8, tag="otlo", name="OTlo")
        # zero the rotary padding rows of the DoubleRow slot-1
        for h in range(GH):
            pad = slice(0, DR_DIM) if h in UPPER else slice(DR_DIM, P)
            nc.any.memset(KKr[pad, h, :, 1, :], 0.0)
            nc.any.memset(QQr[pad, h, 1, :], 0.0)

        # ---------------- pre-phase: K / Kr / V / Q builds (blk-major) ----
        # seqT, wq, bq persist into phase C (Q builds for qb>=1 are emitted
        # just-in-time inside phase C to fill its act-bound slack)
        LO, HI = slice(0, DR_DIM), slice(DR_DIM, P)
        seqp = ctx.enter_context(tc.tile_pool(name="seqp", bufs=1))
        seqT = seqp.tile([P, 32, S], FP8, tag="seqT", name="seqT")
        wq_tiles = []
        for h in range(GH):
            wq = seqp.tile([P, 32, P], FP8, tag="wq", bufs=4, name=f"wq{h}")
            wq_tiles.append(wq)
        bq = seqp.tile([P, GH], F32, tag="bq", name="bq")
        with tc.tile_pool(name="pre_ps", bufs=4, space="PSUM") as pp, \
             tc.tile_pool(name="pre_in", bufs=1) as pin:
            # wdkv rides first on gpsimd in fine-grained pieces so the first
            # C_KV chain starts ASAP; seq blk0 in 8 pieces on sync+scalar
            wdkv = pin.tile([P, 32, 512], FP8, tag="wdkv", name="wdkv")
            for q in range(8):
                nc.gpsimd.dma_start(wdkv[:, 4 * q:4 * (q + 1), :],
                                    t["wdkv"][:, 4 * q:4 * (q + 1), :])
            for i in range(8):
                eng = nc.sync if i % 2 == 0 else nc.scalar
                eng.dma_start(seqT[:, i * 4:(i + 1) * 4, 0:QB],
                              t["seqT"][0, :, i * 4:(i + 1) * 4, :])
            wuk = pin.tile([P, 4, 512], FP8, tag="wuk", name="wuk")
            wuv = pin.tile([P, 4, 512], FP8, tag="wuv", name="wuv")
            nc.gpsimd.dma_start(wuk[:], t["wuk"][:])
            nc.gpsimd.dma_start(wuv[:], t["wuv"][:])
            # pk1's weight + bqr persist into phase C (deferred pk1 builds)
            wqrk = seqp.tile([P, 3, 32, P], FP8, tag="wqrk", name="wqrk")
            for pk in range(3):
                nc.gpsimd.dma_start(wqrk[:, pk], t["wqrk"][pk])
            # seqT blks 1-3 in halves to keep DMA holds short; wq interleaved
            # so Q0 can start once blk0 + wq are in
            for blk in range(1, 4):
                for hh in range(2):
                    eng = nc.scalar if (blk + hh) % 2 else nc.sync
                    eng.dma_start(seqT[:, hh * 16:(hh + 1) * 16,
                                       blk * QB:(blk + 1) * QB],
                                  t["seqT"][blk, :, hh * 16:(hh + 1) * 16, :])
            # wq last: only the Q0-h0 build needs it near the pre-phase end
            for h in range(GH):
                (nc.sync if h % 2 == 0 else nc.scalar).dma_start(
                    wq_tiles[h][:], t["wq"][h])
            nc.gpsimd.dma_start(bq[:], t["bq"][:])
            bqr = seqp.tile([P, GH], F32, tag="bqr", name="bqr")
            nc.gpsimd.dma_start(bqr[:], t["bqr"][:])
            # C8 = fp8(SC * C_KV^T) [latent 512 as 4x128, keys 2048]
            C8 = pin.tile([P, 4, S], FP8, tag="c8", name="C8")
            for blk in range(4):
                sl = slice(blk * QB, (blk + 1) * QB)
                # latent C_KV^T for this key blk
                for lt in range(4):
                    ps = pp.tile([P, QB], F32, tag="ps", name=f"psC{lt}_{blk}")
                    for j in range(16):
                        nc.tensor.matmul(ps[:],
                                         wdkv[:, 2 * j:2 * j + 2,
                                              lt * P:(lt + 1) * P],
                                         seqT[:, 2 * j:2 * j + 2, sl],
                                         start=(j == 0), stop=(j == 15),
                                         perf_mode=DR)
                    nc.scalar.activation(C8[:, lt, sl], ps[:], AFT.Identity,
                                         scale=1.0)
                # K per head from latent: KKr[:, h, kt, 0, :] = K^T[kdim, keys]
                for h in range(GH):
                    ps = pp.tile([P, QB], F32, tag="ps", name=f"psK{h}_{blk}")
                    for u in range(2):
                        nc.tensor.matmul(ps[:],
                                         wuk[:, 2 * u:2 * u + 2,
                                             h * P:(h + 1) * P],
                                         C8[:, 2 * u:2 * u + 2, sl],
                                         start=(u == 0), stop=(u == 1),
                                         perf_mode=DR)
                    nc.vector.tensor_scalar_mul(
                        KKr[:, h, blk * 4:(blk + 1) * 4, 0, :], ps[:],
                        1.0 / (SC * SC2))
                # V from latent: V_G[:, kt, :] = V[keys 128, 512 vdims]
                for k4 in range(4):
                    kt = blk * 4 + k4
                    ps = pp.tile([P, 512], F32, tag="ps", name=f"psV{kt}")
                    for u in range(2):
                        nc.tensor.matmul(ps[:],
                                         C8[:, 2 * u:2 * u + 2,
                                            kt * P:(kt + 1) * P],
                                         wuv[:, 2 * u:2 * u + 2, :],
                                         start=(u == 0), stop=(u == 1),
                                         perf_mode=DR)
                    nc.vector.tensor_scalar_mul(V_G[:, kt, :], ps[:],
                                                1.0 / (SC * SC2))
                # packed Qr/Kr builds: [Kr|Qr_h0], [Qr_h1|Qr_h2], [Qr_h3|Kr];
                # pk1 is pure query-side: blks 1-3 are deferred into qb0's
                # phase C slack (their Qr is first read at qb1)
                for pk in ((0, 2) if blk > 0 else (0, 1, 2)):
                    ps = pp.tile([P, QB], F32, tag="ps", name=f"psP{pk}_{blk}")
                    for j in range(16):
                        nc.tensor.matmul(ps[:], wqrk[:, pk, 2 * j:2 * j + 2, :],
                                         seqT[:, 2 * j:2 * j + 2, sl],
                                         start=(j == 0), stop=(j == 15),
                                         perf_mode=DR)
                    if pk == 0:
                        nc.vector.tensor_scalar_mul(Kr8[LO, sl], ps[LO, :],
                                                    1.0 / SC)
                        nc.scalar.activation(QQr[HI, 0, 1, sl], ps[HI, :],
                                             AFT.Identity,
                                             bias=bqr[HI, 0:1],
                                             scale=SCALER / SC)
                    elif pk == 1:
                        nc.scalar.activation(QQr[LO, 1, 1, sl], ps[LO, :],
                                             AFT.Identity,
                                             bias=bqr[LO, 1:2],
                                             scale=SCALER / SC)
                        nc.scalar.activation(QQr[HI, 2, 1, sl], ps[HI, :],
                                             AFT.Identity,
                                             bias=bqr[HI, 2:3],
                                             scale=SCALER / SC)
                    else:
                        nc.scalar.activation(QQr[LO, 3, 1, sl], ps[LO, :],
                                             AFT.Identity,
                                             bias=bqr[LO, 3:4],
                                             scale=SCALER / SC)
                        nc.vector.tensor_scalar_mul(Kr8[HI, sl], ps[HI, :],
                                                    1.0 / SC)
            # replicate Kr into each head's slot-1 rows
            for h in range(GH):
                half = HI if h in UPPER else LO
                nc.vector.tensor_copy(KKr[half, h, :, 1, :], Kr8[half, :])
            # Q content for (h0, qb0) only -- h1-h3 fill qb0's phase C slack,
            # later qbs are built just-in-time inside phase C
            ps = pp.tile([P, QB], F32, tag="ps", name="psQ0_0")
            for j in range(16):
                nc.tensor.matmul(ps[:], wq_tiles[0][:, 2 * j:2 * j + 2, :],
                                 seqT[:, 2 * j:2 * j + 2, 0:QB],
                                 start=(j == 0), stop=(j == 15), perf_mode=DR)
            nc.scalar.activation(QQr[:, 0, 0, 0:QB], ps[:], AFT.Identity,
                                 bias=bq[:, 0:1], scale=SCALER / SC)

        # ---------------- phase C (+ interleaved phase D) ----------------
        pc = ctx.enter_context(tc.tile_pool(name="pc", bufs=1))
        pdw = ctx.enter_context(tc.tile_pool(name="pdw", bufs=1))
        pss = pod = pou = None  # bound by the phase C `with` below

        wout_cur = {}            # nt -> tile, all 8 stay resident

        def load_wout(nt):
            if nt in wout_cur:
                return wout_cur[nt]
            w = pdw.tile([P, GH, 512], FP8, tag="wout", bufs=8,
                         name=f"wout{nt}")
            nc.sync.dma_start(w[:], t["wout"][nt])
            wout_cur[nt] = w
            return w

        for nt in range(8):
            load_wout(nt)

        d_jobs = []     # pending (qb, nt, qt) out-projection chains
        d_seq = [0]
        q0_tile = [None]
        pk1_tile = [None]
        osb_cur = [None]

        def emit_d_job(pool=None, drain=False):
            if not d_jobs:
                return
            qb, nt, qt = d_jobs.pop(0)
            w = load_wout(nt)
            ps = (pool or pou).tile([P, 512], F32, tag="po",
                                    name=f"psOut{qb}_{nt}_{qt}")
            qsl = slice(qb * QB + qt * P, qb * QB + (qt + 1) * P)
            for i, ot in enumerate((OThi, OTlo)):
                for hp in range(2):
                    nc.tensor.matmul(ps[:], ot[:, 2 * hp:2 * hp + 2, qsl],
                                     w[:, 2 * hp:2 * hp + 2, :],
                                     start=(i == 0 and hp == 0),
                                     stop=(i == 1 and hp == 1),
                                     perf_mode=DR)
            # f16 copy (no scale -- host divides the summed partials by SC^2)
            # into a 4-wide staging tile; jobs run qt-major / nt-minor so 4
            # consecutive tiles form one contiguous DRAM row-block shipped as
            # ONE output DMA (4x fewer issues + semaphores)
            seq_i = d_seq[0] = d_seq[0] + 1
            if osb_cur[0] is None:
                osb_cur[0] = pdw.tile([P, 4, 512], F16, tag="osb", bufs=4,
                                      name=f"osb{qb}_{nt}_{qt}")
            osb = osb_cur[0]
            if drain:
                if seq_i % 2 == 0:
                    nc.vector.tensor_copy(osb[:, nt % 4, :], ps[:])
                else:
                    nc.scalar.activation(osb[:, nt % 4, :], ps[:],
                                         AFT.Identity)
            else:
                # gpsimd cannot read PSUM; DVE does the phase C copies
                nc.vector.tensor_copy(osb[:, nt % 4, :], ps[:])
            if nt % 4 == 3:
                deng = nc.scalar if (drain and seq_i % 2 == 1) else nc.sync
                deng.dma_start(
                    t["pout"][qb * QB + qt * P: qb * QB + (qt + 1) * P,
                              (nt - 3) * 512:(nt + 1) * 512],
                    osb[:])
                osb_cur[0] = None

        # software-pipelined: each iteration's tail (final AV pairs, JIT-Q
        # chain, softmax epilogue) is emitted AFTER the next iteration's
        # first two score pairs so the exp stream never starves at
        # iteration boundaries.
        pending_tail = [None]

        def make_tail(qb, h, PT, psO, psD):
            def tail():
                for j in (6, 7):
                    nc.tensor.matmul(psO[:],
                                     V_G[:, 2 * j:2 * j + 2,
                                         h * P:(h + 1) * P],
                                     PT[:, 2 * j:2 * j + 2, :],
                                     start=False, stop=(j == 7),
                                     perf_mode=DR)
                    nc.tensor.matmul(psD[:], ones8[:],
                                     PT[:, 2 * j:2 * j + 2, :],
                                     start=False, stop=(j == 7),
                                     perf_mode=DR)
                rcp = pc.tile([P, QB], F32, tag="rcp", bufs=2,
                              name=f"rcp{qb}_{h}")
                nc.vector.reciprocal(rcp[:], psD[:])
                # rcp = SC/den, so ots = SC*OT; split into fp8 hi + residual
                sl_q = slice(qb * QB, (qb + 1) * QB)
                ots = pc.tile([P, QB], BF16, tag="ots", bufs=2,
                              name=f"ots{qb}_{h}")
                nc.vector.tensor_tensor(ots[:], psO[:], rcp[:], ALU.mult)
                nc.vector.tensor_copy(OThi[:, h, sl_q], ots[:])
                nc.vector.tensor_tensor(OTlo[:, h, sl_q], ots[:],
                                        OThi[:, h, sl_q], ALU.subtract)
                if qb < 3:
                    # JIT Q-content build for the next query block
                    nsl = slice((qb + 1) * QB, (qb + 2) * QB)
                    psQ = pou.tile([P, QB], F32, tag="po",
                                   name=f"psQj{qb}_{h}")
                    for j in range(16):
                        nc.tensor.matmul(psQ[:],
                                         wq_tiles[h][:, 2 * j:2 * j + 2, :],
                                         seqT[:, 2 * j:2 * j + 2, nsl],
                                         start=(j == 0), stop=(j == 15),
                                         perf_mode=DR)
                    nc.vector.tensor_scalar(QQr[:, h, 0, nsl], psQ[:],
                                            SCALER / SC, bq[:, h:h + 1],
                                            ALU.mult, ALU.add)
                if h == GH - 1:
                    # queue this qb's out-projection, qt-major / nt-minor so
                    # output tiles group into contiguous DRAM row-blocks
                    for qt in range(4):
                        for nt in range(8):
                            d_jobs.append((qb, nt, qt))
            return tail

        with tc.tile_pool(name="pss", bufs=2, space="PSUM") as pss, \
             tc.tile_pool(name="pod", bufs=2, space="PSUM") as pod, \
             tc.tile_pool(name="pou", bufs=2, space="PSUM") as pou:
          for qb in range(4):
            for h in range(GH):
                PT = pc.tile([P, 16, QB], FP8, tag="pt", bufs=2,
                             name=f"pt{qb}_{h}")
                psO = pod.tile([P, QB], F32, tag="pod", name=f"psO{qb}_{h}")
                psD = pod.tile([P, QB], F32, tag="pod", name=f"psD{qb}_{h}")
                for tt in range(8):
                    psS = pss.tile([P, 2, QB], F32, tag="pss",
                                   name=f"psS{qb}_{h}_{tt}")
                    for u in range(2):
                        kt = 2 * tt + u
                        nc.tensor.matmul(psS[:, u, :], KKr[:, h, kt, :, :],
                                         QQr[:, h, :, qb * QB:(qb + 1) * QB],
                                         start=True, stop=True, perf_mode=DR)
                    nc.scalar.activation(PT[:, 2 * tt:2 * tt + 2, :],
                                         psS[:, :, :], AFT.Exp)
                    if tt == 1 and pending_tail[0] is not None:
                        pending_tail[0]()
                        pending_tail[0] = None
                    if tt >= 2:
                        emit_d_job()
                        j = tt - 2
                        nc.tensor.matmul(psO[:],
                                         V_G[:, 2 * j:2 * j + 2,
                                             h * P:(h + 1) * P],
                                         PT[:, 2 * j:2 * j + 2, :],
                                         start=(j == 0), stop=False,
                                         perf_mode=DR)
                        nc.tensor.matmul(psD[:], ones8[:],
                                         PT[:, 2 * j:2 * j + 2, :],
                                         start=(j == 0), stop=False,
                                         perf_mode=DR)
                    if qb == 0 and h < GH - 1 and tt in (3, 5):
                        # qb0 has no D jobs yet: build Q content for the
                        # next head's qb0 scores in its slack (half chains)
                        if tt == 3:
                            psQ0 = pou.tile([P, QB], F32, tag="po",
                                            name=f"psQ0_{h + 1}")
                            q0_tile[0] = psQ0
                        else:
                            psQ0 = q0_tile[0]
                        for j in range(8 * (tt == 5), 8 + 8 * (tt == 5)):
                            nc.tensor.matmul(
                                psQ0[:], wq_tiles[h + 1][:, 2 * j:2 * j + 2, :],
                                seqT[:, 2 * j:2 * j + 2, 0:QB],
                                start=(j == 0), stop=(j == 15), perf_mode=DR)
                        if tt == 5:
                            nc.vector.tensor_scalar(QQr[:, h + 1, 0, 0:QB],
                                                    psQ0[:], SCALER / SC,
                                                    bq[:, h + 1:h + 2],
                                                    ALU.mult, ALU.add)
                    if qb == 0 and h < GH - 1 and tt in (2, 4):
                        # deferred pk1 build for blk h+1 (rotary Q for
                        # heads 1/2), also filling qb0's slack
                        bsl = slice((h + 1) * QB, (h + 2) * QB)
                        if tt == 2:
                            psP = pou.tile([P, QB], F32, tag="po",
                                           name=f"psPj_{h + 1}")
                            pk1_tile[0] = psP
                        else:
                            psP = pk1_tile[0]
                        for j in range(8 * (tt == 4), 8 + 8 * (tt == 4)):
                            nc.tensor.matmul(
                                psP[:], wqrk[:, 1, 2 * j:2 * j + 2, :],
                                seqT[:, 2 * j:2 * j + 2, bsl],
                                start=(j == 0), stop=(j == 15), perf_mode=DR)
                        if tt == 4:
                            nc.vector.tensor_scalar(QQr[LO, 1, 1, bsl],
                                                    psP[LO, :], SCALER / SC,
                                                    bqr[LO, 1:2],
                                                    ALU.mult, ALU.add)
                            nc.vector.tensor_scalar(QQr[HI, 2, 1, bsl],
                                                    psP[HI, :], SCALER / SC,
                                                    bqr[HI, 2:3],
                                                    ALU.mult, ALU.add)
                emit_d_job()
                emit_d_job()
                pending_tail[0] = make_tail(qb, h, PT, psO, psD)
          pending_tail[0]()
        # drain: exp/AV PSUM banks are free now; use a wide pool and both
        # copy engines so the final 32 jobs stream at full PE rate
        with tc.tile_pool(name="pdr", bufs=6, space="PSUM") as pdr:
            while d_jobs:
                emit_d_job(pool=pdr, drain=True)


def _build_program(rep=1):
    nc = bacc.Bacc("TRN2", target_bir_lowering=False, debug=False)
    t = {}
    t["seqT"] = nc.dram_tensor("t_seqT", [4, P, 32, QB], FP8,
                               kind="ExternalInput")
    t["wq"] = nc.dram_tensor("t_wq", [GH, P, 32, P], FP8, kind="ExternalInput")
    t["wqrk"] = nc.dram_tensor("t_wqrk", [3, P, 32, P], FP8,
                               kind="ExternalInput")
    t["wdkv"] = nc.dram_tensor("t_wdkv", [P, 32, 512], FP8,
                               kind="ExternalInput")
    t["wuk"] = nc.dram_tensor("t_wuk", [P, 4, 512], FP8, kind="ExternalInput")
    t["wuv"] = nc.dram_tensor("t_wuv", [P, 4, 512], FP8, kind="ExternalInput")
    t["bq"] = nc.dram_tensor("t_bq", [P, GH], F32, kind="ExternalInput")
    t["bqr"] = nc.dram_tensor("t_bqr", [P, GH], F32, kind="ExternalInput")
    t["wout"] = nc.dram_tensor("t_wout", [8, P, GH, 512], FP8,
                               kind="ExternalInput")
    t["pout"] = nc.dram_tensor("t_pout", [S, DM], F16, kind="ExternalOutput")

    with tile.TileContext(nc) as tc:
        for _ in range(rep):
            _emit_body(nc, tc, t)
    nc.compile()
    return nc


def _prep_in_maps(inputs):
    f32 = np.float32
    seq = np.asarray(inputs["sequence"], dtype=f32)[0]          # [2048, 4096]
    w_dq = np.asarray(inputs["w_dq"], dtype=f32)
    b_dq = np.asarray(inputs["b_dq"], dtype=f32)
    w_uq = np.asarray(inputs["w_uq"], dtype=f32)
    b_uq = np.asarray(inputs["b_uq"], dtype=f32)
    w_dkv = np.asarray(inputs["w_dkv"], dtype=f32)
    b_dkv = np.asarray(inputs["b_dkv"], dtype=f32)
    w_uk = np.asarray(inputs["w_uk"], dtype=f32)
    w_uv = np.asarray(inputs["w_uv"], dtype=f32)
    b_uv = np.asarray(inputs["b_uv"], dtype=f32)
    w_rq = np.asarray(inputs["w_rq"], dtype=f32)
    b_rq = np.asarray(inputs["b_rq"], dtype=f32)
    w_rk = np.asarray(inputs["w_rk"], dtype=f32)
    w_out = np.asarray(inputs["w_out"], dtype=f32)

    W_q = (w_dq @ w_uq) * SC                                    # [4096, 4096]
    W_qr = (w_dq @ w_rq) * SC                                   # [4096, 2048]
    W_kr = w_rk * SC                                            # [4096, 64]
    b_q = (b_dq @ w_uq + b_uq) * SCALER                         # [4096]
    b_qr = (b_dq @ w_rq + b_rq) * SCALER                        # [2048]

    seqT = np.ascontiguousarray(
        seq.reshape(4, QB, 32, P).transpose(0, 3, 2, 1)
    ).astype(NP8)                                               # [4,128,32,512]

    def tile32(w):  # [4096, n] -> [128, 32, n]
        return np.ascontiguousarray(
            w.reshape(32, P, w.shape[1]).transpose(1, 0, 2)).astype(NP8)

    def tile4(w):  # [512, n] -> [128, 4, n]
        return np.ascontiguousarray(
            w.reshape(4, P, w.shape[1]).transpose(1, 0, 2)).astype(NP8)

    shared = {"seqT": seqT, "wdkv": tile32(w_dkv * SC)}
    in_maps = []
    for c in range(8):
        cols = slice(c * GH * DH, (c + 1) * GH * DH)            # 512 cols
        colr = slice(c * GH * DR_DIM, (c + 1) * GH * DR_DIM)    # 256 cols
        m = dict(shared)
        m["wq"] = np.ascontiguousarray(
            W_q[:, cols].reshape(32, P, GH, DH).transpose(2, 1, 0, 3)
        ).astype(NP8)
        # packed Qr/Kr stationary tiles: [Kr|Qr_h0], [Qr_h1|Qr_h2], [Qr_h3|Kr]
        qr = [W_qr[:, colr][:, h * DR_DIM:(h + 1) * DR_DIM] for h in range(GH)]
        m["wqrk"] = np.stack([
            tile32(np.hstack([W_kr, qr[0]])),
            tile32(np.hstack([qr[1], qr[2]])),
            tile32(np.hstack([qr[3], W_kr])),
        ])
        m["wuk"] = tile4(w_uk[:, cols] * SC2)
        m["wuv"] = tile4(w_uv[:, cols] * SC2)
        m["bq"] = np.ascontiguousarray(
            b_q[cols].reshape(GH, P).T, dtype=f32)
        bqr_h = b_qr[colr].reshape(GH, DR_DIM).T                # [64, GH]
        m["bqr"] = np.ascontiguousarray(
            np.vstack([bqr_h, bqr_h]), dtype=f32)               # [128, GH]
        w64 = np.ascontiguousarray(
            (w_out[cols, :] * SC).reshape(GH, P, 8, 512).transpose(2, 1, 0, 3))
        m["wout"] = w64.astype(NP8)
        in_maps.append({f"t_{k}": v for k, v in m.items()})
    return in_maps


class _Runner:
    """Cached sharded PJRT executor for a compiled Bass program."""

    def __init__(self, nc):
        bass2jax.install_neuronx_cc_hook()
        self.nc = nc
        in_names, out_names, out_avals = [], [], []
        pid_name = nc.partition_id_tensor.name if nc.partition_id_tensor else None
        for alloc in nc.m.functions[0].allocations:
            if not isinstance(alloc, mybir.MemoryLocationSet):
                continue
            name = alloc.memorylocations[0].name
            if alloc.kind == "ExternalInput":
                if name != pid_name:
                    in_names.append(name)
            elif alloc.kind == "ExternalOutput":
                out_names.append(name)
                shape = tuple(alloc.tensor_shape)
                dtype = mybir.dt.np(alloc.dtype)
                out_avals.append(jax.core.ShapedArray(shape, dtype))
        self.in_names = in_names
        self.out_names = out_names
        all_in_names = list(in_names) + list(out_names)
        if pid_name is not None:
            all_in_names.append(pid_name)

        def _body(*args):
            operands = list(args)
            if nc.partition_id_tensor is not None:
                operands.append(bass2jax.partition_id_tensor())
            outs = bass2jax._bass_exec_p.bind(
                *operands,
                out_avals=tuple(out_avals),
                in_names=tuple(all_in_names),
                out_names=tuple(out_names),
                lowering_input_output_aliases=(),
                sim_require_finite=True,
                sim_require_nnan=True,
                nc=nc,
            )
            return tuple(outs)

        devices = jax.devices()[:8]
        self.mesh = Mesh(np.asarray(devices), ("core",))
        n_io = len(in_names) + len(out_names)
        self.fn = jax.jit(
            shard_map(_body, mesh=self.mesh,
                      in_specs=(PartitionSpec("core"),) * n_io,
                      out_specs=(PartitionSpec("core"),) * len(out_names),
                      check_rep=False),
            keep_unused=True)
        self.sharding = NamedSharding(self.mesh, PartitionSpec("core"))
        self.dev_zero = [
            jax.device_put(
                np.zeros((8 * av.shape[0], *av.shape[1:]), av.dtype),
                self.sharding)
            for av in out_avals]
        self.out_avals = out_avals

    def stage(self, in_maps):
        dev_in = []
        for name in self.in_names:
            cat = np.concatenate([np.asarray(m[name]) for m in in_maps],
                                 axis=0)
            dev_in.append(jax.device_put(cat, self.sharding))
        return dev_in

    def run_staged(self, dev_in):
        outs = self.fn(*dev_in, *self.dev_zero)
        jax.block_until_ready(outs)
        return outs

    def run(self, in_maps):
        outs = self.run_staged(self.stage(in_maps))
        res = []
        for c in range(8):
            d = {}
            for i, name in enumerate(self.out_names):
                av = self.out_avals[i]
                d[name] = np.asarray(outs[i]).reshape(8, *av.shape)[c]
            res.append(d)
        return res


_CTX = None


def _get_ctx():
    global _CTX
    if _CTX is None:
        nc = _build_program(rep=1)
        _CTX = _Runner(nc)
    return _CTX


def kernel(**inputs):
    runner = _get_ctx()
    in_maps = _prep_in_maps(inputs)
    res = runner.run(in_maps)
    b_out = np.asarray(inputs["b_out"], dtype=np.float32)
    b_dkv = np.asarray(inputs["b_dkv"], dtype=np.float32)
    w_uv = np.asarray(inputs["w_uv"], dtype=np.float32)
    b_uv = np.asarray(inputs["b_uv"], dtype=np.float32)
    w_out = np.asarray(inputs["w_out"], dtype=np.float32)
    b_v = b_dkv @ w_uv + b_uv
    b_out_eff = b_out + b_v @ w_out
    out = res[0]["t_pout"].astype(np.float32)
    for c in range(1, 8):
        out = out + res[c]["t_pout"].astype(np.float32)
    out = out * (1.0 / (SC * SC)) + b_out_eff
    return out.reshape(1, S, DM)

